# revision 10
# baseline (speedup 1.0000x reference)
"""Trainium2 Bass kernel for nn_DecoderBlock (DETR-style decoder block).

Sharding: 900 object queries split across 8 cores (113 each, padded to 904).
Key/value-side projections (k, v over 900 keys; enc projections over 2500)
are replicated on every core — no collectives.

Layouts (on device, all chosen so no transpose instruction is ever needed):
  - activations feature-major:  x^T stored [128, n_ftile, tokens]
  - attention scores TRANSPOSED: s^T [keys, q] so softmax denominators come
    from ones-matmuls and P^T feeds attention matmuls directly
  - v token-major [keys, feat] with an extra ones column per head (the
    attention matmul then produces the softmax denominator as row 64)
  - LayerNorm feature-major: partition-dim stats via ones-matmuls
Host side does all transposes/casts (numpy) before launch.
"""
import sys

sys.path.insert(0, "/opt/trn_rl_repo")

import numpy as np
import ml_dtypes

import concourse.bass as bass
import concourse.mybir as mybir
import concourse.tile as tile
from concourse import bacc

P = 128
NQ = 900
NCORES = 8
NS = 113          # queries per core
NQP = NS * NCORES  # 904
EMB = 512
CH = 256
H = 8
DH = 64
S_ENC = 2500
KT = [(k * P, min(P, NQ - k * P)) for k in range((NQ + P - 1) // P)]      # 8 key tiles
ST = [(s * P, min(P, S_ENC - s * P)) for s in range((S_ENC + P - 1) // P)]  # 20 enc tiles
F32 = mybir.dt.float32
BF = mybir.dt.bfloat16
AF = mybir.ActivationFunctionType
ALU = mybir.AluOpType
SC_SA = 1.0 / np.sqrt(DH)      # 0.125
SC_CA = 1.0 / np.sqrt(EMB)
EPS = 1e-5

_NC_CACHE = {}


def _chunks(n, c):
    out = []
    o = 0
    while o < n:
        out.append((o, min(c, n - o)))
        o += c
    return out


def build_nc(debug=False):
    nc = bacc.Bacc()

    def din(name, shape, dt=BF):
        return nc.dram_tensor(name, shape, dt, kind="ExternalInput")

    # ---- weights (shared across cores) ----
    w_q = din("w_q", [EMB, EMB]); w_k = din("w_k", [EMB, EMB]); w_v = din("w_v", [EMB, EMB])
    w_qp = din("w_qp", [CH, CH]); w_kp = din("w_kp", [CH, CH])
    w_osa = din("w_osa", [EMB, EMB]); w_opa = din("w_opa", [EMB, EMB])
    w_cq = din("w_cq", [EMB, EMB]); w_cqp = din("w_cqp", [CH, CH])
    w_ke = din("w_ke", [CH, CH]); w_kpe = din("w_kpe", [CH, CH]); w_ve = din("w_ve", [CH, CH])
    b_osa = din("b_osa", [EMB], F32); b_opa = din("b_opa", [EMB], F32)
    ln_g1 = din("ln_g1", [EMB], F32); ln_b12 = din("ln_b12", [EMB], F32)  # 0.5*g1, 0.5*(b1+b2)
    ln_g2 = din("ln_g2", [EMB], F32)
    wo_b = [din(f"wo_{x}", [CH, CH]) for x in "cr"]
    bo_b = [din(f"bo_{x}", [CH], F32) for x in "cr"]
    w1_b = [din(f"w1_{x}", [CH, 4 * CH]) for x in "cr"]
    b1_b = [din(f"b1_{x}", [4 * CH], F32) for x in "cr"]
    w2_b = [din(f"w2_{x}", [4 * CH, CH]) for x in "cr"]
    b2_b = [din(f"b2_{x}", [CH], F32) for x in "cr"]
    n1g_b = [din(f"n1g_{x}", [CH], F32) for x in "cr"]
    n1b_b = [din(f"n1b_{x}", [CH], F32) for x in "cr"]
    n2g_b = [din(f"n2g_{x}", [CH], F32) for x in "cr"]
    n2b_b = [din(f"n2b_{x}", [CH], F32) for x in "cr"]

    # ---- activations (shared full + per-core slices) ----
    oqT = din("oqT", [EMB, NQP])            # object_queries^T (padded)
    posT = din("posT", [CH, NQ])            # obj_pos_embed^T (keys side, exact 900)
    encT = din("encT", [CH, S_ENC])
    peT = din("peT", [CH, S_ENC])
    oq_sT = din("oq_sT", [EMB, NS])         # per-core query slice (bf16)
    oq_sT32 = din("oq_sT32", [EMB, NS], F32)
    pos_sT = din("pos_sT", [CH, NS])
    sin_sT = din("sin_sT", [CH, NS])
    m2cT = din("m2cT", [2, NQ], F32)        # -2 * coords^T (keys)
    c_sT = din("c_sT", [2, NS], F32)        # per-core coords^T
    csq = din("csq", [1, NS], F32)          # per-core |coords|^2 row
    coordsQ = din("coordsQ", [len(KT) * P, 2], F32)  # key coords padded to 1024

    outT = nc.dram_tensor("outT", [EMB, NS], F32, kind="ExternalOutput")
    dbg = {}
    if debug:
        for nm, shp in [("d_kT", [EMB, NQ]), ("d_qT", [EMB, NS]), ("d_oT", [EMB, NS]),
                        ("d_v", [len(KT) * P, EMB]), ("d_att1", [EMB, NS])]:
            dbg[nm] = nc.dram_tensor(nm, shp, F32, kind="ExternalOutput")

    with tile.TileContext(nc) as tc:
        with (
            tc.tile_pool(name="sg", bufs=1) as sg,        # singles / persistents
            tc.tile_pool(name="wk", bufs=3) as wk,        # rotating work tiles
            tc.tile_pool(name="psA", bufs=3, space="PSUM") as psA,
            tc.tile_pool(name="psB", bufs=4, space="PSUM") as psB,
        ):
            def ld(handle, shape, pat, tag, dt=BF, **kw):
                t = sg.tile(shape, dt, tag=tag)
                src = handle.rearrange(pat, **kw) if pat else handle[:, :]
                nc.sync.dma_start(t, src)
                return t

            # weights into SBUF, lhsT layout [128, ktile, fout]
            W_q = ld(w_q, [P, 4, EMB], "(kt p) m -> p kt m", "W_q", p=P)
            W_k = ld(w_k, [P, 4, EMB], "(kt p) m -> p kt m", "W_k", p=P)
            W_v = ld(w_v, [P, 4, EMB], "(kt p) m -> p kt m", "W_v", p=P)
            W_qp = ld(w_qp, [P, 2, CH], "(kt p) m -> p kt m", "W_qp", p=P)
            W_kp = ld(w_kp, [P, 2, CH], "(kt p) m -> p kt m", "W_kp", p=P)
            W_osa = ld(w_osa, [P, 4, EMB], "(kt p) m -> p kt m", "W_osa", p=P)
            W_opa = ld(w_opa, [P, 4, EMB], "(kt p) m -> p kt m", "W_opa", p=P)
            W_cq = ld(w_cq, [P, 4, EMB], "(kt p) m -> p kt m", "W_cq", p=P)
            W_cqp = ld(w_cqp, [P, 2, CH], "(kt p) m -> p kt m", "W_cqp", p=P)
            W_ke = ld(w_ke, [P, 2, CH], "(kt p) m -> p kt m", "W_ke", p=P)
            W_kpe = ld(w_kpe, [P, 2, CH], "(kt p) m -> p kt m", "W_kpe", p=P)
            W_ve = ld(w_ve, [P, 2, CH], "(kt p) m -> p kt m", "W_ve", p=P)
            Wo_b = [ld(wo_b[i], [P, 2, CH], "(kt p) m -> p kt m", f"Wo{i}", p=P) for i in range(2)]
            W1_b = [ld(w1_b[i], [P, 2, 4 * CH], "(kt p) m -> p kt m", f"W1{i}", p=P) for i in range(2)]
            W2_b = [ld(w2_b[i], [P, 8, CH], "(kt p) m -> p kt m", f"W2{i}", p=P) for i in range(2)]

            def ldcol(handle, nf, tag):
                return ld(handle, [P, nf], "(f p) -> p f", tag, dt=F32, p=P)

            B_osa = ldcol(b_osa, 4, "B_osa"); B_opa = ldcol(b_opa, 4, "B_opa")
            G1 = ldcol(ln_g1, 4, "G1"); G2 = ldcol(ln_g2, 4, "G2"); B12 = ldcol(ln_b12, 4, "B12")
            Bo_b = [ldcol(bo_b[i], 2, f"Bo{i}") for i in range(2)]
            B1_b = [ldcol(b1_b[i], 8, f"B1{i}") for i in range(2)]
            B2_b = [ldcol(b2_b[i], 2, f"B2{i}") for i in range(2)]
            N1g = [ldcol(n1g_b[i], 2, f"N1g{i}") for i in range(2)]
            N1b = [ldcol(n1b_b[i], 2, f"N1b{i}") for i in range(2)]
            N2g = [ldcol(n2g_b[i], 2, f"N2g{i}") for i in range(2)]
            N2b = [ldcol(n2b_b[i], 2, f"N2b{i}") for i in range(2)]

            OQ = ld(oqT, [P, 4, NQP], "(kt p) q -> p kt q", "OQ", p=P)
            POS = ld(posT, [P, 2, NQ], "(kt p) q -> p kt q", "POS", p=P)
            ENC = ld(encT, [P, 2, S_ENC], "(kt p) q -> p kt q", "ENC", p=P)
            PE = ld(peT, [P, 2, S_ENC], "(kt p) q -> p kt q", "PE", p=P)
            OQS = ld(oq_sT, [P, 4, NS], "(kt p) q -> p kt q", "OQS", p=P)
            OQS32 = ld(oq_sT32, [P, 4, NS], "(kt p) q -> p kt q", "OQS32", dt=F32, p=P)
            POSS = ld(pos_sT, [P, 2, NS], "(kt p) q -> p kt q", "POSS", p=P)
            SINS = ld(sin_sT, [P, 2, NS], "(kt p) q -> p kt q", "SINS", p=P)
            M2C = ld(m2cT, [2, NQ], None, "M2C", dt=F32)
            CST = ld(c_sT, [2, NS], None, "CST", dt=F32)
            CQ = ld(coordsQ, [P, len(KT), 2], "(kt p) c -> p kt c", "CQ", dt=F32, p=P)
            # csq broadcast to all 128 partitions via stride-0 DMA
            NQB = sg.tile([P, NS], F32, tag="NQB")
            csq_ap = csq[0, :]
            nc.sync.dma_start(NQB, bass.AP(tensor=csq_ap.tensor, offset=csq_ap.offset,
                                           ap=[[0, P]] + [list(x) for x in csq_ap.ap]))

            ones_b = sg.tile([P, P], BF, tag="ones_b")
            nc.vector.memset(ones_b, 1.0)
            ones1f = sg.tile([1, P], F32, tag="ones1f")
            nc.vector.memset(ones1f, 1.0)
            epsc = sg.tile([P, 1], F32, tag="epsc")
            nc.vector.memset(epsc, EPS)

            # ================= replicated projections =================
            # k^T feature-major [128, 4, 900] with k_pos folded in
            kposT = sg.tile([P, 2, NQ], BF, tag="kposT")
            for f in range(2):
                for c0, cn in _chunks(NQ, 450):
                    ps = psA.tile([P, 512], F32, tag="psA")
                    for kt2 in range(2):
                        nc.tensor.matmul(ps[:, :cn], W_kp[:, kt2, f * P:(f + 1) * P],
                                         POS[:, kt2, c0:c0 + cn],
                                         start=(kt2 == 0), stop=(kt2 == 1))
                    nc.vector.tensor_copy(kposT[:, f, c0:c0 + cn], ps[:, :cn])
            KTm = sg.tile([P, 4, NQ], BF, tag="KTm")
            for f in range(4):
                for c0, cn in _chunks(NQ, 450):
                    ps = psA.tile([P, 512], F32, tag="psA")
                    for kt2 in range(4):
                        nc.tensor.matmul(ps[:, :cn], W_k[:, kt2, f * P:(f + 1) * P],
                                         OQ[:, kt2, c0:c0 + cn],
                                         start=(kt2 == 0), stop=(kt2 == 3))
                    nc.vector.tensor_add(KTm[:, f, c0:c0 + cn], ps[:, :cn],
                                         kposT[:, f % 2, c0:c0 + cn])
            # v token-major with ones column: [128, kt, head, 65]
            V65 = sg.tile([P, len(KT), H, DH + 1], BF, tag="V65")
            for kt2, (k0, kn) in enumerate(KT):
                ps = psA.tile([P, 512], F32, tag="psA")
                for ft in range(4):
                    nc.tensor.matmul(ps[:kn, :], OQ[:, ft, k0:k0 + kn], W_v[:, ft, :],
                                     start=(ft == 0), stop=(ft == 3))
                nc.scalar.copy(V65[:kn, kt2, :, 0:DH],
                               ps[:kn, :].rearrange("p (h d) -> p h d", h=H))
                nc.vector.memset(V65[:kn, kt2, :, DH:DH + 1], 1.0)
            # enc-side: k_enc^T, k_pe^T fm [128,2,2500]; v2 token-major [128,20,256]
            KE = sg.tile([P, 2, S_ENC], BF, tag="KE")
            KPE = sg.tile([P, 2, S_ENC], BF, tag="KPE")
            for f in range(2):
                for c0, cn in _chunks(S_ENC, 500):
                    ps = psA.tile([P, 512], F32, tag="psA")
                    for kt2 in range(2):
                        nc.tensor.matmul(ps[:, :cn], W_ke[:, kt2, f * P:(f + 1) * P],
                                         ENC[:, kt2, c0:c0 + cn],
                                         start=(kt2 == 0), stop=(kt2 == 1))
                    nc.scalar.copy(KE[:, f, c0:c0 + cn], ps[:, :cn])
                for c0, cn in _chunks(S_ENC, 500):
                    ps = psA.tile([P, 512], F32, tag="psA")
                    for kt2 in range(2):
                        nc.tensor.matmul(ps[:, :cn], W_kpe[:, kt2, f * P:(f + 1) * P],
                                         PE[:, kt2, c0:c0 + cn],
                                         start=(kt2 == 0), stop=(kt2 == 1))
                    nc.scalar.copy(KPE[:, f, c0:c0 + cn], ps[:, :cn])
            V2 = sg.tile([P, len(ST), CH], BF, tag="V2")
            for st, (s0, sn) in enumerate(ST):
                ps = psA.tile([P, 512], F32, tag="psA")
                for ft in range(2):
                    nc.tensor.matmul(ps[:sn, :CH], ENC[:, ft, s0:s0 + sn], W_ve[:, ft, :],
                                     start=(ft == 0), stop=(ft == 1))
                nc.vector.tensor_copy(V2[:sn, st, :], ps[:sn, :CH])

            # ================= q projections (per-core shard) =================
            qposT = sg.tile([P, 2, NS], BF, tag="qposT")
            for f in range(2):
                ps = psA.tile([P, 512], F32, tag="psA")
                for kt2 in range(2):
                    nc.tensor.matmul(ps[:, :NS], W_qp[:, kt2, f * P:(f + 1) * P],
                                     POSS[:, kt2, :], start=(kt2 == 0), stop=(kt2 == 1))
                nc.vector.tensor_copy(qposT[:, f, :], ps[:, :NS])
            QT = sg.tile([P, 4, NS], BF, tag="QT")
            for f in range(4):
                ps = psA.tile([P, 512], F32, tag="psA")
                for kt2 in range(4):
                    nc.tensor.matmul(ps[:, :NS], W_q[:, kt2, f * P:(f + 1) * P],
                                     OQS[:, kt2, :], start=(kt2 == 0), stop=(kt2 == 3))
                nc.vector.tensor_add(QT[:, f, :], ps[:, :NS], qposT[:, f % 2, :])

            # ================= pairwise distance -> exp(-dist) =================
            # nk = |coord_k|^2 + 1e-6 per key partition
            csqK = sg.tile([P, len(KT), 2], F32, tag="csqK")
            nc.vector.tensor_mul(csqK, CQ, CQ)
            NK = sg.tile([P, len(KT)], F32, tag="NK")
            nc.vector.tensor_reduce(NK, csqK, axis=mybir.AxisListType.X, op=ALU.add)
            nc.vector.tensor_scalar_add(NK, NK, 1e-6)
            EXND = sg.tile([P, len(KT), NS], F32, tag="EXND")
            for kt2, (k0, kn) in enumerate(KT):
                ps = psA.tile([P, 512], F32, tag="psA")
                nc.tensor.matmul(ps[:kn, :NS], M2C[:, k0:k0 + kn], CST, start=True, stop=True)
                d2 = wk.tile([P, NS], F32, tag="d2", bufs=2)
                nc.vector.tensor_add(d2[:kn], ps[:kn, :NS], NQB[:kn])
                dst = wk.tile([P, NS], F32, tag="dst", bufs=2)
                nc.scalar.activation(dst[:kn], d2[:kn], AF.Sqrt, bias=NK[:kn, kt2:kt2 + 1])
                nc.scalar.activation(EXND[:kn, kt2, :], dst[:kn], AF.Exp, scale=-1.0)

            # ================= self attention (per head, transposed scores) ========
            oat = [sg.tile([P, 4, NS], BF, tag=f"oat{i}", name=f"oat{i}") for i in range(2)]  # attn out fm
            for h in range(H):
                ft, off = h // 2, (h % 2) * DH
                po = [psB.tile([P, P], F32, tag="psB", name="po") for _ in range(2)]
                for kt2, (k0, kn) in enumerate(KT):
                    ps = psA.tile([P, 512], F32, tag="psA")
                    nc.tensor.matmul(ps[:kn, :NS], KTm[off:off + DH, ft, k0:k0 + kn],
                                     QT[off:off + DH, ft, :], start=True, stop=True)
                    p1 = wk.tile([P, NS], BF, tag="p1")
                    nc.scalar.activation(p1[:kn], ps[:kn, :NS], AF.Exp, scale=SC_SA)
                    p2 = wk.tile([P, NS], BF, tag="p2")
                    nc.vector.tensor_mul(p2[:kn], p1[:kn], EXND[:kn, kt2, :])
                    for i, pp in enumerate((p1, p2)):
                        nc.tensor.matmul(po[i][:DH + 1, :NS], V65[:kn, kt2, h, :], pp[:kn],
                                         start=(kt2 == 0), stop=(kt2 == len(KT) - 1))
                for i in range(2):
                    r1 = wk.tile([1, NS], F32, tag="r1")
                    nc.vector.reciprocal(r1, po[i][DH:DH + 1, :NS])
                    pb = psA.tile([P, 512], F32, tag="psA")
                    nc.tensor.matmul(pb[:, :NS], ones1f, r1, start=True, stop=True)
                    ib = wk.tile([P, NS], F32, tag="ib", bufs=2)
                    nc.vector.tensor_copy(ib[:DH], pb[:DH, :NS])
                    nc.vector.tensor_mul(oat[i][off:off + DH, ft, :], po[i][:DH, :NS], ib[:DH])

            # ================= Wo + residual + dual LN fusion =================
            xT = [sg.tile([P, 4, NS], F32, tag=f"xT{i}", name=f"xT{i}") for i in range(2)]
            for i, (Wo_, Bo_) in enumerate(((W_osa, B_osa), (W_opa, B_opa))):
                for f in range(4):
                    ps = psA.tile([P, 512], F32, tag="psA")
                    for kt2 in range(4):
                        nc.tensor.matmul(ps[:, :NS], Wo_[:, kt2, f * P:(f + 1) * P],
                                         oat[i][:, kt2, :], start=(kt2 == 0), stop=(kt2 == 3))
                    nc.vector.scalar_tensor_tensor(xT[i][:, f, :], ps[:, :NS],
                                                   Bo_[:, f:f + 1], OQS32[:, f, :],
                                                   op0=ALU.add, op1=ALU.add)

            def ln_stats(x32, nf, tag):
                """x32: [128, nf, NS] f32 -> (mean_bcast, rstd_bcast) [128, NS] f32."""
                xb = wk.tile([P, nf, NS], BF, tag="ln_xb", bufs=2, name="xb")
                nc.scalar.copy(xb, x32)
                xs = wk.tile([P, nf, NS], BF, tag="ln_xs", bufs=2, name="xs")
                nc.vector.tensor_mul(xs, x32, x32)
                pm = psA.tile([P, 512], F32, tag="psA")
                for f in range(nf):
                    nc.tensor.matmul(pm[:, :NS], ones_b, xb[:, f, :],
                                     start=(f == 0), stop=(f == nf - 1))
                pv = psA.tile([P, 512], F32, tag="psA")
                for f in range(nf):
                    nc.tensor.matmul(pv[:, :NS], ones_b, xs[:, f, :],
                                     start=(f == 0), stop=(f == nf - 1))
                m = wk.tile([P, NS], F32, tag="ln_m", bufs=2, name="m")
                nc.scalar.mul(m, pm[:, :NS], 1.0 / (nf * P))
                e2 = wk.tile([P, NS], F32, tag="ln_e2", bufs=2, name="e2")
                nc.scalar.mul(e2, pv[:, :NS], 1.0 / (nf * P))
                msq = wk.tile([P, NS], F32, tag="ln_msq", bufs=2, name="msq")
                nc.vector.tensor_mul(msq, m, m)
                var = wk.tile([P, NS], F32, tag="ln_var", bufs=2, name="var")
                nc.vector.tensor_tensor(var, e2, msq, ALU.subtract)
                sd = wk.tile([P, NS], F32, tag="ln_sd", bufs=2, name="sd")
                nc.scalar.activation(sd, var, AF.Sqrt, bias=epsc)
                rs = wk.tile([P, NS], F32, tag="ln_rs", bufs=2, name="rs")
                nc.vector.reciprocal(rs, sd)
                return m, rs

            m1, rs1 = ln_stats(xT[0], 4, "l1")
            m2, rs2 = ln_stats(xT[1], 4, "l2")
            OT32 = sg.tile([P, 4, NS], F32, tag="OT32")
            OTb = sg.tile([P, 4, NS], BF, tag="OTb")
            for f in range(4):
                t1 = wk.tile([P, NS], F32, tag="t1", bufs=2)
                nc.vector.tensor_tensor(t1, xT[0][:, f, :], m1, ALU.subtract)
                nc.vector.tensor_mul(t1, t1, rs1)
                t2 = wk.tile([P, NS], F32, tag="t2", bufs=2)
                nc.vector.tensor_tensor(t2, xT[1][:, f, :], m2, ALU.subtract)
                nc.vector.tensor_mul(t2, t2, rs2)
                acc = wk.tile([P, NS], F32, tag="acc", bufs=2)
                nc.vector.tensor_scalar(acc, t1, G1[:, f:f + 1], B12[:, f:f + 1],
                                        ALU.mult, ALU.add)
                nc.vector.scalar_tensor_tensor(OT32[:, f, :], t2, G2[:, f:f + 1], acc,
                                               op0=ALU.mult, op1=ALU.add)
                nc.scalar.copy(OTb[:, f, :], OT32[:, f, :])

            # ================= cross attention projections =================
            QC = sg.tile([P, 4, NS], BF, tag="QC")
            for f in range(4):
                ps = psA.tile([P, 512], F32, tag="psA")
                for kt2 in range(4):
                    nc.tensor.matmul(ps[:, :NS], W_cq[:, kt2, f * P:(f + 1) * P],
                                     OTb[:, kt2, :], start=(kt2 == 0), stop=(kt2 == 3))
                nc.vector.tensor_copy(QC[:, f, :], ps[:, :NS])
            QS = sg.tile([P, 2, NS], BF, tag="QS")
            for f in range(2):
                ps = psA.tile([P, 512], F32, tag="psA")
                for kt2 in range(2):
                    nc.tensor.matmul(ps[:, :NS], W_cqp[:, kt2, f * P:(f + 1) * P],
                                     SINS[:, kt2, :], start=(kt2 == 0), stop=(kt2 == 1))
                nc.vector.tensor_copy(QS[:, f, :], ps[:, :NS])

            # ================= per-branch cross attention + FFN =================
            OUT = sg.tile([P, 4, NS], F32, tag="OUT")
            for bi in range(2):  # 0=cls, 1=reg
                pdn = psB.tile([P, P], F32, tag="psB")
                pco = [psB.tile([P, P], F32, tag="psB", name="pco") for _ in range(2)]
                for st, (s0, sn) in enumerate(ST):
                    ps = psA.tile([P, 512], F32, tag="psA")
                    for mt in range(2):
                        nc.tensor.matmul(ps[:sn, :NS], KE[:, mt, s0:s0 + sn],
                                         QC[:, bi * 2 + mt, :], start=(mt == 0), stop=False)
                    for mt in range(2):
                        nc.tensor.matmul(ps[:sn, :NS], KPE[:, mt, s0:s0 + sn],
                                         QS[:, mt, :], start=False, stop=(mt == 1))
                    pc = wk.tile([P, NS], BF, tag="pc")
                    nc.scalar.activation(pc[:sn], ps[:sn, :NS], AF.Exp, scale=SC_CA)
                    last = (st == len(ST) - 1)
                    nc.tensor.matmul(pdn[:, :NS], ones_b[:sn, :], pc[:sn],
                                     start=(st == 0), stop=last)
                    for mt in range(2):
                        nc.tensor.matmul(pco[mt][:, :NS], V2[:sn, st, mt * P:(mt + 1) * P],
                                         pc[:sn], start=(st == 0), stop=last)
                dn = wk.tile([P, NS], F32, tag="dn", bufs=2)
                nc.vector.reciprocal(dn, pdn[:, :NS])
                cao = wk.tile([P, 2, NS], BF, tag="cao", bufs=2)
                for mt in range(2):
                    nc.vector.tensor_mul(cao[:, mt, :], pco[mt][:, :NS], dn)
                # Wo + bias + residual(o branch half) -> x
                xc = wk.tile([P, 2, NS], F32, tag="xc", bufs=2)
                for mt in range(2):
                    ps = psA.tile([P, 512], F32, tag="psA")
                    for kt2 in range(2):
                        nc.tensor.matmul(ps[:, :NS], Wo_b[bi][:, kt2, mt * P:(mt + 1) * P],
                                         cao[:, kt2, :], start=(kt2 == 0), stop=(kt2 == 1))
                    nc.vector.scalar_tensor_tensor(xc[:, mt, :], ps[:, :NS],
                                                   Bo_b[bi][:, mt:mt + 1],
                                                   OT32[:, bi * 2 + mt, :],
                                                   op0=ALU.add, op1=ALU.add)
                mc, rc = ln_stats(xc, 2, "lc")
                xln = wk.tile([P, 2, NS], F32, tag="xln", bufs=2)
                xlb = wk.tile([P, 2, NS], BF, tag="xlb", bufs=2)
                for mt in range(2):
                    t = wk.tile([P, NS], F32, tag="tc", bufs=2)
                    nc.vector.tensor_tensor(t, xc[:, mt, :], mc, ALU.subtract)
                    nc.vector.tensor_mul(t, t, rc)
                    nc.vector.tensor_scalar(xln[:, mt, :], t, N1g[bi][:, mt:mt + 1],
                                            N1b[bi][:, mt:mt + 1], ALU.mult, ALU.add)
                    nc.scalar.copy(xlb[:, mt, :], xln[:, mt, :])
                # FFN
                hb = wk.tile([P, 8, NS], BF, tag="hb", bufs=2)
                for mt in range(8):
                    ps = psA.tile([P, 512], F32, tag="psA")
                    for kt2 in range(2):
                        nc.tensor.matmul(ps[:, :NS], W1_b[bi][:, kt2, mt * P:(mt + 1) * P],
                                         xlb[:, kt2, :], start=(kt2 == 0), stop=(kt2 == 1))
                    nc.scalar.activation(hb[:, mt, :], ps[:, :NS], AF.Relu,
                                         bias=B1_b[bi][:, mt:mt + 1])
                yc = wk.tile([P, 2, NS], F32, tag="yc", bufs=2)
                for mt in range(2):
                    ps = psA.tile([P, 512], F32, tag="psA")
                    for kt2 in range(8):
                        nc.tensor.matmul(ps[:, :NS], W2_b[bi][:, kt2, mt * P:(mt + 1) * P],
                                         hb[:, kt2, :], start=(kt2 == 0), stop=(kt2 == 7))
                    nc.vector.scalar_tensor_tensor(yc[:, mt, :], ps[:, :NS],
                                                   B2_b[bi][:, mt:mt + 1], xln[:, mt, :],
                                                   op0=ALU.add, op1=ALU.add)
                my, ry = ln_stats(yc, 2, "ly")
                for mt in range(2):
                    t = wk.tile([P, NS], F32, tag="ty", bufs=2)
                    nc.vector.tensor_tensor(t, yc[:, mt, :], my, ALU.subtract)
                    nc.vector.tensor_mul(t, t, ry)
                    nc.vector.tensor_scalar(OUT[:, bi * 2 + mt, :], t,
                                            N2g[bi][:, mt:mt + 1], N2b[bi][:, mt:mt + 1],
                                            ALU.mult, ALU.add)

            nc.sync.dma_start(outT.rearrange("(f p) q -> p f q", p=P), OUT)

            if debug:
                def dump(name, t, f32shape, nf):
                    tmp = wk.tile([P, nf, f32shape[1]], F32, tag="dumpbuf", bufs=1, name="dumpbuf")
                    nc.vector.tensor_copy(tmp, t)
                    nc.sync.dma_start(dbg[name].rearrange("(f p) q -> p f q", p=P), tmp)
                dump("d_kT", KTm, [EMB, NQ], 4)
                dump("d_qT", QT, [EMB, NS], 4)
                dump("d_oT", OT32, [EMB, NS], 4)
                dump("d_att1", oat[0], [EMB, NS], 4)
                vtmp = wk.tile([P, len(KT), H, DH], F32, tag="dumpbuf", bufs=1, name="dumpbuf")
                nc.vector.tensor_copy(vtmp, V65[:, :, :, 0:DH])
                nc.sync.dma_start(
                    dbg["d_v"].rearrange("(kt p) (h d) -> p kt h d", p=P, h=H), vtmp)

    nc.finalize()
    return nc


def _bf(x):
    return np.ascontiguousarray(x.astype(ml_dtypes.bfloat16))


def _f32(x):
    return np.ascontiguousarray(x.astype(np.float32))


def prep_inputs(object_queries, encoder_output, obj_coords, obj_pos_embed,
                obj_sin_embed, pe2d, sa, ca, cls_p, reg_p):
    """Host-side prep: transposes, casts, padding, per-core slices."""
    oq = np.asarray(object_queries, np.float32)[0]      # [900, 512]
    enc = np.asarray(encoder_output, np.float32)[0]     # [2500, 256]
    crd = np.asarray(obj_coords, np.float32)[0]         # [900, 2]
    pos = np.asarray(obj_pos_embed, np.float32)[0]      # [900, 256]
    sin = np.asarray(obj_sin_embed, np.float32)[0]      # [900, 256]
    pe = np.asarray(pe2d, np.float32)[0]                # [2500, 256]
    sa = {k: np.asarray(v, np.float32) for k, v in sa.items()}
    ca = {k: np.asarray(v, np.float32) for k, v in ca.items()}
    brs = [{k: np.asarray(v, np.float32) for k, v in p.items()} for p in (cls_p, reg_p)]

    def padq(x):  # [900, d] -> [904, d]
        return np.concatenate([x, np.zeros((NQP - NQ,) + x.shape[1:], x.dtype)], 0)

    oqp = padq(oq); posp = padq(pos); sinp = padq(sin); crdp = padq(crd)
    shared = {
        "w_q": _bf(sa["Wq_obj"]), "w_k": _bf(sa["Wk_obj"]), "w_v": _bf(sa["Wv_obj"]),
        "w_qp": _bf(sa["Wq_pos"]), "w_kp": _bf(sa["Wk_pos"]),
        "w_osa": _bf(sa["Wo_sa"]), "w_opa": _bf(sa["Wo_pa"]),
        "b_osa": _f32(sa["bo_sa"]), "b_opa": _f32(sa["bo_pa"]),
        "ln_g1": _f32(0.5 * sa["ln1_g"]), "ln_g2": _f32(0.5 * sa["ln2_g"]),
        "ln_b12": _f32(0.5 * (sa["ln1_b"] + sa["ln2_b"])),
        "w_cq": _bf(ca["Wq_obj"]), "w_cqp": _bf(ca["Wq_pos"]),
        "w_ke": _bf(ca["Wk_enc"]), "w_kpe": _bf(ca["Wk_pos"]), "w_ve": _bf(ca["Wv_enc"]),
        "oqT": _bf(oqp.T), "posT": _bf(pos.T), "encT": _bf(enc.T), "peT": _bf(pe.T),
        "m2cT": _f32(-2.0 * crd.T),
        "coordsQ": _f32(np.concatenate([crd, np.zeros((len(KT) * P - NQ, 2), np.float32)], 0)),
    }
    for i, x in enumerate("cr"):
        b = brs[i]
        shared.update({
            f"wo_{x}": _bf(b["Wo"]), f"bo_{x}": _f32(b["bo"]),
            f"w1_{x}": _bf(b["W1"]), f"b1_{x}": _f32(b["b1"]),
            f"w2_{x}": _bf(b["W2"]), f"b2_{x}": _f32(b["b2"]),
            f"n1g_{x}": _f32(b["n1_g"]), f"n1b_{x}": _f32(b["n1_b"]),
            f"n2g_{x}": _f32(b["n2_g"]), f"n2b_{x}": _f32(b["n2_b"]),
        })
    in_maps = []
    for c in range(NCORES):
        s = slice(c * NS, (c + 1) * NS)
        m = dict(shared)
        m["oq_sT"] = _bf(oqp[s].T); m["oq_sT32"] = _f32(oqp[s].T)
        m["pos_sT"] = _bf(posp[s].T); m["sin_sT"] = _bf(sinp[s].T)
        m["c_sT"] = _f32(crdp[s].T)
        m["csq"] = _f32((crdp[s] ** 2).sum(-1)[None, :])
        in_maps.append(m)
    return in_maps


def run(inputs, trace=False, debug=False):
    from concourse.bass_utils import run_bass_kernel_spmd
    key = debug
    if key not in _NC_CACHE:
        _NC_CACHE[key] = build_nc(debug=debug)
    nc = _NC_CACHE[key]
    in_maps = prep_inputs(**inputs)
    res = run_bass_kernel_spmd(nc, in_maps, core_ids=list(range(NCORES)), trace=trace)
    outs = np.concatenate([np.asarray(r["outT"]) for r in res.results], axis=1)
    full = outs[:, :NQ].T[None]  # [1, 900, 512]
    return np.ascontiguousarray(full, dtype=np.float32), res


def kernel(**inputs):
    out, _ = run(inputs, trace=False)
    return out
